# revision 19
# baseline (speedup 1.0000x reference)
"""Multi-head attention block (q/k/v projections + softmax attention +
out-projection) distributed over 8 TRN2 NeuronCores.

Sharding: core c handles batch b = c//2 and query rows [h*1024, (h+1)*1024),
h = c%2. Each core keeps the full kv of its batch (kv projections are
recomputed per query-half) so no inter-core collective is needed; the full
output is assembled host-side from disjoint shards.

Per-core dataflow:
  q/kv -> bf16 DRAM scratch (one contiguous SWDGE cast-DMA each)
       -> qT/kvT [model_dim, seq] in SBUF via HWDGE DMA-transpose
  projections (bf16 matmuls, fp32 PSUM):
       qhT/khT [inner, seq] transposed layout (stored fp32r for the score
       matmuls), vh [seq_k, head*(64+1)] natural bf16 with a ones column
       (P@[V|1] then yields the softmax denominator for free)
  attention per head-pair (two heads row-packed in the PE via tile_position,
  contraction dim HEAD_DIM=64):
       scores S^T[k, q] on PSUM (fp32r) -> exp(s/8) fused on ScalarE -> bf16
       -> PV [65, q] PSUM accumulation over the 16 k tiles (bf16)
       -> denominator row -> Kc=1 ones-matmul broadcast -> fast reciprocal
       -> multiply -> attnT (fp32r)
  out-projection (fp32r): per-pair partial matmuls accumulated into an SBUF
  fp32 buffer (bias folded in at the first partial), streamed into the
  attention phase as PE filler.

The attention k-loop is software-pipelined as [scores(k+1) | filler | PV(k)]
so the TensorEngine never waits on ScalarE's exp, never idles long enough
for the HAM clock gate to re-throttle it, and the leftover projection /
out-projection work rides in the bubbles.
"""

import sys

sys.path.insert(0, "/opt/trn_rl_repo")

import numpy as np

B, NQ_FULL, NK = 4, 2048, 2048
NQ = 1024          # per-core query rows
DQ, DKV = 512, 768
HEADS, DH = 8, 64
INNER = 512
DA = DH + 1        # head dim + ones column
N_CORES = 8

_cache = {}


def _build():
    import concourse.bass as bass
    import concourse.tile as tile
    from concourse import bacc, mybir

    F32 = mybir.dt.float32
    F32R = mybir.dt.float32r
    BF16 = mybir.dt.bfloat16
    EXP = mybir.ActivationFunctionType.Exp

    nc = bacc.Bacc("TRN2", target_bir_lowering=False, debug=False,
                   enable_asserts=True, num_devices=N_CORES)

    q_d = nc.dram_tensor("q", [NQ, DQ], F32, kind="ExternalInput").ap()
    kv_d = nc.dram_tensor("kv", [NK, DKV], F32, kind="ExternalInput").ap()
    wq_d = nc.dram_tensor("Wq", [DQ, INNER], F32, kind="ExternalInput").ap()
    wk_d = nc.dram_tensor("Wk", [DKV, INNER], F32, kind="ExternalInput").ap()
    wv_d = nc.dram_tensor("Wv", [DKV, INNER], F32, kind="ExternalInput").ap()
    wo_d = nc.dram_tensor("Wo", [INNER, DQ], F32, kind="ExternalInput").ap()
    bo_d = nc.dram_tensor("bo", [DQ], F32, kind="ExternalInput").ap()
    out_d = nc.dram_tensor("out", [NQ, DQ], F32, kind="ExternalOutput").ap()

    MT_Q = DQ // 128      # 4
    MT_KV = DKV // 128    # 6
    IT = INNER // 128     # 4 inner tiles (= head pairs)
    KT = NK // 128        # 16
    QB = NQ // 512        # 2
    NT = NQ // 128        # 8 output row tiles
    PAIRS = HEADS // 2    # 4

    with tile.TileContext(nc) as tc:
        with (
            tc.tile_pool(name="consts", bufs=1) as consts,
            tc.tile_pool(name="wpool", bufs=1) as wpool,
            tc.tile_pool(name="xT", bufs=1) as xT_pool,
            tc.tile_pool(name="proj", bufs=1) as proj_pool,
            tc.tile_pool(name="attnT", bufs=1) as attnT_pool,
            tc.tile_pool(name="oacc", bufs=1) as oacc_pool,
            tc.tile_pool(name="exps", bufs=4) as exps_pool,
            tc.tile_pool(name="norm", bufs=2) as norm_pool,
            tc.tile_pool(name="outs", bufs=2) as outs_pool,
            tc.tile_pool(name="mm", bufs=2, space="PSUM") as ps_mm,
            tc.tile_pool(name="sc", bufs=2, space="PSUM") as ps_sc,
            tc.tile_pool(name="pv", bufs=2, space="PSUM") as ps_pv,
        ):
            # ---- PE warm-up: ~4us of throwaway matmuls so the HAM clock
            # gate reaches K=8/8 (2.4 GHz) before the transposes and first
            # projections run (nothing else occupies the PE this early).
            from concourse.masks import make_identity
            ident = consts.tile([128, 128], F32)
            make_identity(nc, ident[:])
            ident_bf = consts.tile([128, 128], BF16)
            nc.vector.tensor_copy(ident_bf[:], ident[:])
            junk = consts.tile([128, 512], BF16)
            nc.vector.memset(junk[:], 1.0)
            jp = ps_mm.tile([128, 512], F32, tag="mm", name="jp")
            for _ in range(36):
                nc.tensor.matmul(jp[:, 0:256], junk[:, 0:128], junk[:, 0:256],
                                 start=True, stop=True)

            wk_b = wpool.tile([128, MT_KV, 512], BF16, tag="wk")
            wq_b = wpool.tile([128, MT_Q, 512], BF16, tag="wq")
            wv_b = wpool.tile([128, MT_KV, 512], BF16, tag="wv")
            wo_r = wpool.tile([128, IT, 512], F32R, tag="wo")
            kvT = [xT_pool.tile([128, NK], BF16, tag=f"kvT{mt}", name=f"kvT{mt}")
                   for mt in range(MT_KV)]
            qT = [xT_pool.tile([128, NQ], BF16, tag=f"qT{mt}", name=f"qT{mt}")
                  for mt in range(MT_Q)]

            from contextlib import ExitStack
            _es = ExitStack()
            wstage = _es.enter_context(tc.tile_pool(name="wstage", bufs=1))
            kvnat_pool = _es.enter_context(tc.tile_pool(name="kvnat", bufs=2))

            def load_w(wd, wt, mt):
                st = wstage.tile([128, mt, 512], F32, tag="wst", name="wst")
                nc.sync.dma_start(st[:], wd.rearrange("(t p) i -> p t i", p=128))
                nc.vector.tensor_copy(wt[:], st[:])

            # kv arrives in 8 groups of 2 k-tiles; groups 0..3 are staged and
            # transposed before attention starts, groups 4..7 stream into the
            # first attention block as PE filler.
            kvgrp = [None] * 8

            def load_kv(g):
                kn = kvnat_pool.tile([128, 2, DKV], F32, tag="kn", name="kn")
                nc.sync.dma_start(
                    kn[:],
                    kv_d[g * 256:(g + 1) * 256, :]
                    .rearrange("(t p) i -> p t i", p=128))
                knb = kvnat_pool.tile([128, 2, DKV], BF16, tag="knb", name="knb")
                nc.vector.tensor_copy(knb[:], kn[:])
                kvgrp[g] = knb

            def tr_kv(g, mts):
                knb = kvgrp[g]
                for mt in mts:
                    pt = ps_mm.tile([128, 256], BF16, tag="mm", name="pt")
                    for j in range(2):
                        nc.tensor.transpose(
                            pt[:, j * 128:(j + 1) * 128],
                            knb[:, j, mt * 128:(mt + 1) * 128],
                            ident_bf[:])
                    nc.vector.tensor_copy(
                        kvT[mt][:, g * 256:(g + 1) * 256], pt[:])

            load_kv(0)
            load_kv(1)
            load_w(wk_d, wk_b, MT_KV)
            load_kv(2)
            load_kv(3)
            load_w(wv_d, wv_b, MT_KV)
            for g in range(4):
                tr_kv(g, range(MT_KV))
            load_w(wq_d, wq_b, MT_Q)
            with tc.tile_pool(name="qnat", bufs=2) as qnat_pool:
                for g in range(2):
                    qn = qnat_pool.tile([128, 4, DQ], F32, tag="qn", name="qn")
                    nc.sync.dma_start(
                        qn[:],
                        q_d[g * 512:(g + 1) * 512, :]
                        .rearrange("(t p) i -> p t i", p=128))
                    qnb = qnat_pool.tile([128, 4, DQ], BF16, tag="qnb", name="qnb")
                    nc.vector.tensor_copy(qnb[:], qn[:])
                    for mt in range(MT_Q):
                        pt = ps_mm.tile([128, 512], BF16, tag="mm", name="pt")
                        for j in range(4):
                            nc.tensor.transpose(
                                pt[:, j * 128:(j + 1) * 128],
                                qnb[:, j, mt * 128:(mt + 1) * 128],
                                ident_bf[:])
                        nc.vector.tensor_copy(
                            qT[mt][:, g * 512:(g + 1) * 512], pt[:])
            for g in range(4, 8):
                load_kv(g)
            load_w(wo_d, wo_r, IT)

            # ---- constants ----
            ones1f = consts.tile([1, 64], F32)
            nc.vector.memset(ones1f[:], 1.0)
            ones1 = consts.tile([1, 64], F32R)
            nc.vector.tensor_copy(ones1[:], ones1f[:])
            ones8 = consts.tile([128, 8, 1], BF16)
            ones8f = consts.tile([128, 8, 1], F32)
            nc.vector.memset(ones8f[:], 1.0)
            nc.vector.tensor_copy(ones8[:], ones8f[:])
            bo_b = consts.tile([128, DQ], F32)
            nc.gpsimd.dma_start(
                out=bo_b[:],
                in_=bass.AP(tensor=bo_d.tensor, offset=bo_d.offset,
                            ap=[[0, 128]] + list(bo_d.ap)),
            )

            # ---- projection outputs / accumulators ----
            qhT = [proj_pool.tile([128, NQ], BF16, tag=f"qhT{i}", name=f"qhT{i}")
                   for i in range(IT)]
            khT = [proj_pool.tile([128, NK], BF16, tag=f"khT{i}", name=f"khT{i}")
                   for i in range(IT)]
            vh = [proj_pool.tile([128, HEADS, DA], BF16, tag=f"vh{k}", name=f"vh{k}")
                  for k in range(KT)]
            attnT = [attnT_pool.tile([128, NQ], F32R, tag=f"at{i}", name=f"at{i}")
                     for i in range(IT)]
            oacc = [oacc_pool.tile([128, DQ], F32, tag=f"oa{nt}", name=f"oa{nt}")
                    for nt in range(NT)]

            def emit_khT(it, nb):
                pp = ps_mm.tile([128, 512], F32, tag="mm", name="pp")
                for mt in range(MT_KV):
                    nc.tensor.matmul(
                        pp[:], wk_b[:, mt, it * 128:(it + 1) * 128],
                        kvT[mt][:, nb * 512:(nb + 1) * 512],
                        start=(mt == 0), stop=(mt == MT_KV - 1))
                nc.vector.tensor_copy(khT[it][:, nb * 512:(nb + 1) * 512], pp[:])

            def emit_qhT(it, nb):
                pp = ps_mm.tile([128, 512], F32, tag="mm", name="pp")
                for mt in range(MT_Q):
                    nc.tensor.matmul(
                        pp[:], wq_b[:, mt, it * 128:(it + 1) * 128],
                        qT[mt][:, nb * 512:(nb + 1) * 512],
                        start=(mt == 0), stop=(mt == MT_Q - 1))
                nc.vector.tensor_copy(qhT[it][:, nb * 512:(nb + 1) * 512], pp[:])

            def emit_vh(kt):
                pp = ps_mm.tile([128, 512], F32, tag="mm", name="pp")
                for mt in range(MT_KV):
                    nc.tensor.matmul(
                        pp[:], kvT[mt][:, kt * 128:(kt + 1) * 128],
                        wv_b[:, mt, :],
                        start=(mt == 0), stop=(mt == MT_KV - 1))
                nc.vector.tensor_copy(
                    vh[kt][:, :, 0:DH],
                    pp[:].rearrange("p (h d) -> p h d", h=HEADS))
                nc.vector.tensor_copy(vh[kt][:, :, DH:DA], ones8[:])

            def emit_opart(t, nt):
                # out-projection partial for head pair t, row tile nt:
                # oacc[nt] (+)= attnT[t][:, ns].T @ Wo[pair t rows]
                ns = slice(nt * 128, (nt + 1) * 128)
                po = ps_mm.tile([128, 512], F32, tag="mm", name="po")
                nc.tensor.matmul(po[:], attnT[t][:, ns], wo_r[:, t, :],
                                 start=True, stop=True)
                if t == 0:
                    nc.vector.tensor_add(oacc[nt][:], po[:], bo_b[:])
                else:
                    nc.vector.tensor_add(oacc[nt][:], po[:], oacc[nt][:])

            def emit_final(nt):
                # last out-projection partial (pair 3) + bias-carrying
                # accumulator -> output row tile store
                ns = slice(nt * 128, (nt + 1) * 128)
                po = ps_mm.tile([128, 512], F32, tag="mm", name="po")
                nc.tensor.matmul(po[:], attnT[3][:, ns], wo_r[:, 3, :],
                                 start=True, stop=True)
                ot = outs_pool.tile([128, DQ], F32, tag="ot", name="ot")
                nc.vector.tensor_add(ot[:], po[:], oacc[nt][:])
                nc.sync.dma_start(out_d[ns, :], ot[:])

            # pre-attention minimum: pair-0 projections over kv groups 0..3
            # (k rows 0..1023) and the matching v tiles; the rest streams in
            # as filler
            emit_khT(0, 0)
            emit_khT(0, 1)
            emit_qhT(0, 0)
            for kt in range(6):
                emit_vh(kt)

            # PE filler queues per (pair, q-block). Deps: kv group g feeds
            # khT columns g*256.. and vh[2g..2g+1]; khT(0, nb) is consumed by
            # scores(kt=4nb..) of block (0, 0); opart(t, nt) needs norm(t,
            # nt//4) which runs at the start of the next block after that.
            fillers = {
                (0, 0): ([(lambda g=g, h=h: tr_kv(g, range(3 * h, 3 * h + 3)))
                          for g in (4, 5) for h in (0, 1)]
                         + [lambda: emit_khT(0, 2)]
                         + [(lambda kt=kt: emit_vh(kt)) for kt in (6, 7)]
                         + [(lambda g=g, h=h: tr_kv(g, range(3 * h, 3 * h + 3)))
                            for g in (6, 7) for h in (0, 1)]
                         + [lambda: emit_khT(0, 3)]
                         + [(lambda kt=kt: emit_vh(kt)) for kt in range(8, KT)]
                         + [lambda: emit_qhT(0, 1)]),
                (0, 1): ([(lambda nb=nb: emit_khT(1, nb)) for nb in range(4)]
                         + [(lambda nb=nb: emit_qhT(1, nb)) for nb in range(QB)]),
                (1, 0): ([(lambda nt=nt: emit_opart(0, nt)) for nt in range(4)]
                         + [(lambda nb=nb: emit_khT(2, nb)) for nb in range(2)]),
                (1, 1): ([(lambda nb=nb: emit_khT(2, nb)) for nb in range(2, 4)]
                         + [(lambda nb=nb: emit_qhT(2, nb)) for nb in range(QB)]
                         + [(lambda nt=nt: emit_opart(0, nt)) for nt in range(4, 6)]),
                (2, 0): ([(lambda nt=nt: emit_opart(0, nt)) for nt in range(6, 8)]
                         + [(lambda nb=nb: emit_khT(3, nb)) for nb in range(2)]
                         + [(lambda nt=nt: emit_opart(1, nt)) for nt in range(2)]),
                (2, 1): ([(lambda nb=nb: emit_khT(3, nb)) for nb in range(2, 4)]
                         + [(lambda nb=nb: emit_qhT(3, nb)) for nb in range(QB)]
                         + [(lambda nt=nt: emit_opart(1, nt)) for nt in range(2, 4)]),
                (3, 0): ([(lambda nt=nt: emit_opart(1, nt)) for nt in range(4, 8)]
                         + [(lambda nt=nt: emit_opart(2, nt)) for nt in range(2)]),
                (3, 1): ([(lambda nt=nt: emit_opart(2, nt)) for nt in range(2, 8)]
                         + [(lambda nt=nt: emit_final(nt)) for nt in range(2)]),
            }

            carry = [None]

            def make_norm(t, qb, pvA, pvB):
                def emit():
                    qs = slice(qb * 512, (qb + 1) * 512)
                    dsb = norm_pool.tile([1, 1024], F32R, tag="nrm", name="dsb")
                    nc.vector.tensor_copy(dsb[0:1, 0:512], pvA[DH:DA, :])
                    nc.vector.tensor_copy(dsb[0:1, 512:1024], pvB[DH:DA, :])
                    dba = ps_mm.tile([64, 512], F32, tag="mm", name="dba")
                    dbb = ps_mm.tile([64, 512], F32, tag="mm", name="dbb")
                    nc.tensor.matmul(dba[:], ones1[:], dsb[0:1, 0:512],
                                     start=True, stop=True)
                    nc.tensor.matmul(dbb[:], ones1[:], dsb[0:1, 512:1024],
                                     start=True, stop=True)
                    rb = norm_pool.tile([64, 1024], F32, tag="nrm", name="rb")
                    nc.vector.reciprocal_approx_fast(rb[:, 0:512], dba[:])
                    nc.vector.reciprocal_approx_fast(rb[:, 512:1024], dbb[:])
                    nc.vector.tensor_mul(attnT[t][0:64, qs],
                                         pvA[0:DH, :], rb[:, 0:512])
                    nc.vector.tensor_mul(attnT[t][64:128, qs],
                                         pvB[0:DH, :], rb[:, 512:1024])
                return emit

            # ---- attention, software-pipelined [scores(k+1) | filler | PV(k)]
            for t in range(PAIRS):
                hA, hB = 2 * t, 2 * t + 1
                for qb in range(QB):
                    qs = slice(qb * 512, (qb + 1) * 512)
                    todo = fillers[(t, qb)]
                    fi = 0

                    def emit_scores(kt):
                        ks = slice(kt * 128, (kt + 1) * 128)
                        sc = ps_sc.tile([128, 1024], F32, tag="sc", name="sc")
                        nc.tensor.matmul(
                            sc[:, 0:512],
                            khT[t][0:64, ks], qhT[t][0:64, qs],
                            start=True, stop=True, tile_position=(0, 0))
                        nc.tensor.matmul(
                            sc[:, 512:1024],
                            khT[t][64:128, ks], qhT[t][64:128, qs],
                            start=True, stop=True, tile_position=(64, 0))
                        ex = exps_pool.tile([128, 1024], BF16, tag="exp", name="ex")
                        nc.scalar.activation(ex[:], sc[:], EXP,
                                             scale=float(DH) ** -0.5)
                        return ex

                    # depth-2 software pipeline: PV(k) trails scores(k) by
                    # two iterations; the last two PVs + normalization of this
                    # block carry into the next block's prologue so ScalarE is
                    # never starved at block boundaries.
                    exq = [emit_scores(0)]
                    if carry[0]:
                        carry[0][0]()      # PV(14) of the previous block
                    exq.append(emit_scores(1))
                    if carry[0]:
                        carry[0][1]()      # PV(15) of the previous block
                        carry[0][2]()      # normalization (frees old pv tiles)
                        carry[0] = None
                    pvA = ps_pv.tile([DA, 512], F32, tag="pv", name="pvA")
                    pvB = ps_pv.tile([DA, 512], F32, tag="pv", name="pvB")

                    def mk_pv(kt, ex, pvA=pvA, pvB=pvB, hA=hA, hB=hB):
                        def emit():
                            nc.tensor.matmul(pvA[:], vh[kt][:, hA, :],
                                             ex[:, 0:512],
                                             start=(kt == 0), stop=(kt == KT - 1))
                            nc.tensor.matmul(pvB[:], vh[kt][:, hB, :],
                                             ex[:, 512:1024],
                                             start=(kt == 0), stop=(kt == KT - 1))
                        return emit

                    per_iter = 2 if (t, qb) == (0, 0) else 1
                    for kt in range(2, KT):
                        exq.append(emit_scores(kt))
                        for _ in range(per_iter):
                            if fi < len(todo) and (per_iter == 2 or kt % 2 == 0):
                                todo[fi]()
                                fi += 1
                        mk_pv(kt - 2, exq[kt - 2])()
                    mk_pv(KT - 2, exq[KT - 2])()
                    while fi < len(todo):
                        todo[fi]()
                        fi += 1
                    carry[0] = [mk_pv(KT - 1, exq[KT - 1]), lambda: None,
                                make_norm(t, qb, pvA, pvB)]
                if t == 0:
                    _es.close()   # free the kv staging + weight staging SBUF
            carry[0][0]()
            carry[0][2]()

            # ---- remaining final out-projection rows ----
            for nt in range(2, NT):
                emit_final(nt)

    nc.compile()
    return nc


def kernel(q, kv, Wq, Wk, Wv, Wo, bo):
    from concourse.bass_utils import run_bass_kernel_spmd

    q = np.asarray(q, dtype=np.float32)
    kv = np.asarray(kv, dtype=np.float32)
    Wq = np.ascontiguousarray(np.asarray(Wq, dtype=np.float32))
    Wk = np.ascontiguousarray(np.asarray(Wk, dtype=np.float32))
    Wv = np.ascontiguousarray(np.asarray(Wv, dtype=np.float32))
    Wo = np.ascontiguousarray(np.asarray(Wo, dtype=np.float32))
    bo = np.ascontiguousarray(np.asarray(bo, dtype=np.float32))

    if "nc" not in _cache:
        _cache["nc"] = _build()
    nc = _cache["nc"]

    in_maps = []
    for c in range(N_CORES):
        b, h = c // 2, c % 2
        in_maps.append({
            "q": np.ascontiguousarray(q[b, h * NQ:(h + 1) * NQ]),
            "kv": np.ascontiguousarray(kv[b]),
            "Wq": Wq, "Wk": Wk, "Wv": Wv, "Wo": Wo, "bo": bo,
        })
    res = run_bass_kernel_spmd(nc, in_maps, core_ids=list(range(N_CORES)))
    out = np.empty((B, NQ_FULL, DQ), dtype=np.float32)
    for c in range(N_CORES):
        b, h = c // 2, c % 2
        out[b, h * NQ:(h + 1) * NQ] = res.results[c]["out"]
    return out


# revision 20
# speedup vs baseline: 1.0184x; 1.0184x over previous
"""Multi-head attention block (q/k/v projections + softmax attention +
out-projection) distributed over 8 TRN2 NeuronCores.

Sharding: core c handles batch b = c//2 and query rows [h*1024, (h+1)*1024),
h = c%2. Each core keeps the full kv of its batch (kv projections are
recomputed per query-half) so no inter-core collective is needed; the full
output is assembled host-side from disjoint shards.

Per-core dataflow:
  ~4us of throwaway matmuls first so the PE's HAM clock gate reaches full
  clock before real work arrives.
  q/kv stream in k-groups: load f32 -> DVE cast bf16 -> PE transpose (bf16,
  1 cyc/row) -> qT/kvT [model_dim, seq] bf16 in SBUF. Attention starts once
  half of kv is staged; the rest streams in as PE filler.
  projections (bf16 matmuls, fp32 PSUM):
       qhT/khT [inner, seq] transposed layout bf16, vh [seq_k, head*(64+1)]
       natural bf16 with a ones column (P@[V|1] then yields the softmax
       denominator for free)
  attention per head-pair (two heads row-packed in the 128-row PE array via
  tile_position, contraction dim HEAD_DIM=64):
       scores S^T[k, q] on fp32 PSUM -> exp(s/8) fused on ScalarE -> bf16
       -> PV [65, q] PSUM accumulation over the 16 k tiles
       -> denominator row -> Kc=1 ones-matmul broadcast -> fast reciprocal
       -> multiply -> attnT (fp32r; no max-subtraction needed: logits are
       O(5) so exp is safe in fp32)
  out-projection (fp32r): per-pair partial matmuls accumulated into an SBUF
  fp32 buffer (bias folded into the first partial), streamed into the
  attention phase as PE filler.

The attention k-loop is software-pipelined depth-2 ([scores(k) | filler |
PV(k-2)]) and the last PV + normalization of each block carry into the next
block's prologue, so neither the TensorEngine nor ScalarE stalls at block
boundaries; leftover projection / out-projection / kv-streaming work rides
in the bubbles as filler.
"""

import sys

sys.path.insert(0, "/opt/trn_rl_repo")

import numpy as np

B, NQ_FULL, NK = 4, 2048, 2048
NQ = 1024          # per-core query rows
DQ, DKV = 512, 768
HEADS, DH = 8, 64
INNER = 512
DA = DH + 1        # head dim + ones column
N_CORES = 8

_cache = {}


def _build():
    import concourse.bass as bass
    import concourse.tile as tile
    from concourse import bacc, mybir

    F32 = mybir.dt.float32
    F32R = mybir.dt.float32r
    BF16 = mybir.dt.bfloat16
    EXP = mybir.ActivationFunctionType.Exp

    nc = bacc.Bacc("TRN2", target_bir_lowering=False, debug=False,
                   enable_asserts=True, num_devices=N_CORES)

    q_d = nc.dram_tensor("q", [NQ, DQ], F32, kind="ExternalInput").ap()
    kv_d = nc.dram_tensor("kv", [NK, DKV], F32, kind="ExternalInput").ap()
    wq_d = nc.dram_tensor("Wq", [DQ, INNER], F32, kind="ExternalInput").ap()
    wk_d = nc.dram_tensor("Wk", [DKV, INNER], F32, kind="ExternalInput").ap()
    wv_d = nc.dram_tensor("Wv", [DKV, INNER], F32, kind="ExternalInput").ap()
    wo_d = nc.dram_tensor("Wo", [INNER, DQ], F32, kind="ExternalInput").ap()
    bo_d = nc.dram_tensor("bo", [DQ], F32, kind="ExternalInput").ap()
    out_d = nc.dram_tensor("out", [NQ, DQ], F32, kind="ExternalOutput").ap()

    MT_Q = DQ // 128      # 4
    MT_KV = DKV // 128    # 6
    IT = INNER // 128     # 4 inner tiles (= head pairs)
    KT = NK // 128        # 16
    QB = NQ // 512        # 2
    NT = NQ // 128        # 8 output row tiles
    PAIRS = HEADS // 2    # 4

    with tile.TileContext(nc) as tc:
        with (
            tc.tile_pool(name="consts", bufs=1) as consts,
            tc.tile_pool(name="wpool", bufs=1) as wpool,
            tc.tile_pool(name="xT", bufs=1) as xT_pool,
            tc.tile_pool(name="proj", bufs=1) as proj_pool,
            tc.tile_pool(name="attnT", bufs=1) as attnT_pool,
            tc.tile_pool(name="oacc", bufs=1) as oacc_pool,
            tc.tile_pool(name="exps", bufs=4) as exps_pool,
            tc.tile_pool(name="norm", bufs=2) as norm_pool,
            tc.tile_pool(name="outs", bufs=2) as outs_pool,
            tc.tile_pool(name="mm", bufs=2, space="PSUM") as ps_mm,
            tc.tile_pool(name="sc", bufs=2, space="PSUM") as ps_sc,
            tc.tile_pool(name="pv", bufs=2, space="PSUM") as ps_pv,
        ):
            # ---- PE warm-up: ~4us of throwaway matmuls so the HAM clock
            # gate reaches K=8/8 (2.4 GHz) before the transposes and first
            # projections run (nothing else occupies the PE this early).
            from concourse.masks import make_identity
            ident = consts.tile([128, 128], F32)
            make_identity(nc, ident[:])
            ident_bf = consts.tile([128, 128], BF16)
            nc.vector.tensor_copy(ident_bf[:], ident[:])
            junk = consts.tile([128, 512], BF16)
            nc.vector.memset(junk[:], 1.0)
            jp = ps_mm.tile([128, 512], F32, tag="mm", name="jp")
            for _ in range(36):
                nc.tensor.matmul(jp[:, 0:256], junk[:, 0:128], junk[:, 0:256],
                                 start=True, stop=True)

            wk_b = wpool.tile([128, MT_KV, 512], BF16, tag="wk")
            wq_b = wpool.tile([128, MT_Q, 512], BF16, tag="wq")
            wv_b = wpool.tile([128, MT_KV, 512], BF16, tag="wv")
            wo_r = wpool.tile([128, IT, 512], F32R, tag="wo")
            kvT = [xT_pool.tile([128, NK], BF16, tag=f"kvT{mt}", name=f"kvT{mt}")
                   for mt in range(MT_KV)]
            qT = [xT_pool.tile([128, NQ], BF16, tag=f"qT{mt}", name=f"qT{mt}")
                  for mt in range(MT_Q)]

            from contextlib import ExitStack
            _es = ExitStack()
            wstage = _es.enter_context(tc.tile_pool(name="wstage", bufs=1))
            kvnat_pool = _es.enter_context(tc.tile_pool(name="kvnat", bufs=2))

            def load_w(wd, wt, mt):
                st = wstage.tile([128, mt, 512], F32, tag="wst", name="wst")
                nc.sync.dma_start(st[:], wd.rearrange("(t p) i -> p t i", p=128))
                nc.vector.tensor_copy(wt[:], st[:])

            # kv arrives in 8 groups of 2 k-tiles; groups 0..3 are staged and
            # transposed before attention starts, groups 4..7 stream into the
            # first attention block as PE filler.
            kvgrp = [None] * 8

            def load_kv(g):
                kn = kvnat_pool.tile([128, 2, DKV], F32, tag="kn", name="kn")
                nc.sync.dma_start(
                    kn[:],
                    kv_d[g * 256:(g + 1) * 256, :]
                    .rearrange("(t p) i -> p t i", p=128))
                knb = kvnat_pool.tile([128, 2, DKV], BF16, tag="knb", name="knb")
                nc.vector.tensor_copy(knb[:], kn[:])
                kvgrp[g] = knb

            def tr_kv(g, mts):
                knb = kvgrp[g]
                for mt in mts:
                    pt = ps_mm.tile([128, 256], BF16, tag="mm", name="pt")
                    for j in range(2):
                        nc.tensor.transpose(
                            pt[:, j * 128:(j + 1) * 128],
                            knb[:, j, mt * 128:(mt + 1) * 128],
                            ident_bf[:])
                    nc.vector.tensor_copy(
                        kvT[mt][:, g * 256:(g + 1) * 256], pt[:])

            load_kv(0)
            load_kv(1)
            load_w(wk_d, wk_b, MT_KV)
            load_kv(2)
            load_kv(3)
            load_w(wv_d, wv_b, MT_KV)
            for g in range(4):
                tr_kv(g, range(MT_KV))
            load_w(wq_d, wq_b, MT_Q)
            with tc.tile_pool(name="qnat", bufs=2) as qnat_pool:
                for g in range(2):
                    qn = qnat_pool.tile([128, 4, DQ], F32, tag="qn", name="qn")
                    nc.sync.dma_start(
                        qn[:],
                        q_d[g * 512:(g + 1) * 512, :]
                        .rearrange("(t p) i -> p t i", p=128))
                    qnb = qnat_pool.tile([128, 4, DQ], BF16, tag="qnb", name="qnb")
                    nc.vector.tensor_copy(qnb[:], qn[:])
                    for mt in range(MT_Q):
                        pt = ps_mm.tile([128, 512], BF16, tag="mm", name="pt")
                        for j in range(4):
                            nc.tensor.transpose(
                                pt[:, j * 128:(j + 1) * 128],
                                qnb[:, j, mt * 128:(mt + 1) * 128],
                                ident_bf[:])
                        nc.vector.tensor_copy(
                            qT[mt][:, g * 512:(g + 1) * 512], pt[:])
            for g in range(4, 8):
                load_kv(g)
            load_w(wo_d, wo_r, IT)

            # ---- constants ----
            ones1f = consts.tile([1, 64], F32)
            nc.vector.memset(ones1f[:], 1.0)
            ones1 = consts.tile([1, 64], F32R)
            nc.vector.tensor_copy(ones1[:], ones1f[:])
            ones8 = consts.tile([128, 8, 1], BF16)
            ones8f = consts.tile([128, 8, 1], F32)
            nc.vector.memset(ones8f[:], 1.0)
            nc.vector.tensor_copy(ones8[:], ones8f[:])
            bo_b = consts.tile([128, DQ], F32)
            nc.gpsimd.dma_start(
                out=bo_b[:],
                in_=bass.AP(tensor=bo_d.tensor, offset=bo_d.offset,
                            ap=[[0, 128]] + list(bo_d.ap)),
            )

            # ---- projection outputs / accumulators ----
            qhT = [proj_pool.tile([128, NQ], BF16, tag=f"qhT{i}", name=f"qhT{i}")
                   for i in range(IT)]
            khT = [proj_pool.tile([128, NK], BF16, tag=f"khT{i}", name=f"khT{i}")
                   for i in range(IT)]
            vh = [proj_pool.tile([128, HEADS, DA], BF16, tag=f"vh{k}", name=f"vh{k}")
                  for k in range(KT)]
            attnT = [attnT_pool.tile([128, NQ], F32R, tag=f"at{i}", name=f"at{i}")
                     for i in range(IT)]
            oacc = [oacc_pool.tile([128, DQ], F32, tag=f"oa{nt}", name=f"oa{nt}")
                    for nt in range(NT)]

            def emit_khT(it, nb):
                pp = ps_mm.tile([128, 512], F32, tag="mm", name="pp")
                for mt in range(MT_KV):
                    nc.tensor.matmul(
                        pp[:], wk_b[:, mt, it * 128:(it + 1) * 128],
                        kvT[mt][:, nb * 512:(nb + 1) * 512],
                        start=(mt == 0), stop=(mt == MT_KV - 1))
                nc.vector.tensor_copy(khT[it][:, nb * 512:(nb + 1) * 512], pp[:])

            def emit_qhT(it, nb):
                pp = ps_mm.tile([128, 512], F32, tag="mm", name="pp")
                for mt in range(MT_Q):
                    nc.tensor.matmul(
                        pp[:], wq_b[:, mt, it * 128:(it + 1) * 128],
                        qT[mt][:, nb * 512:(nb + 1) * 512],
                        start=(mt == 0), stop=(mt == MT_Q - 1))
                nc.vector.tensor_copy(qhT[it][:, nb * 512:(nb + 1) * 512], pp[:])

            def emit_vh(kt):
                pp = ps_mm.tile([128, 512], F32, tag="mm", name="pp")
                for mt in range(MT_KV):
                    nc.tensor.matmul(
                        pp[:], kvT[mt][:, kt * 128:(kt + 1) * 128],
                        wv_b[:, mt, :],
                        start=(mt == 0), stop=(mt == MT_KV - 1))
                nc.vector.tensor_copy(
                    vh[kt][:, :, 0:DH],
                    pp[:].rearrange("p (h d) -> p h d", h=HEADS))
                nc.vector.tensor_copy(vh[kt][:, :, DH:DA], ones8[:])

            def emit_opart(t, nt):
                # out-projection partial for head pair t, row tile nt:
                # oacc[nt] (+)= attnT[t][:, ns].T @ Wo[pair t rows]
                ns = slice(nt * 128, (nt + 1) * 128)
                po = ps_mm.tile([128, 512], F32, tag="mm", name="po")
                nc.tensor.matmul(po[:], attnT[t][:, ns], wo_r[:, t, :],
                                 start=True, stop=True)
                if t == 0:
                    nc.vector.tensor_add(oacc[nt][:], po[:], bo_b[:])
                else:
                    nc.vector.tensor_add(oacc[nt][:], po[:], oacc[nt][:])

            def emit_final(nt):
                # last out-projection partial (pair 3) + bias-carrying
                # accumulator -> output row tile store
                ns = slice(nt * 128, (nt + 1) * 128)
                po = ps_mm.tile([128, 512], F32, tag="mm", name="po")
                nc.tensor.matmul(po[:], attnT[3][:, ns], wo_r[:, 3, :],
                                 start=True, stop=True)
                ot = outs_pool.tile([128, DQ], F32, tag="ot", name="ot")
                nc.vector.tensor_add(ot[:], po[:], oacc[nt][:])
                nc.sync.dma_start(out_d[ns, :], ot[:])

            # pre-attention minimum: pair-0 projections over kv groups 0..3
            # (k rows 0..1023) and the matching v tiles; the rest streams in
            # as filler
            emit_khT(0, 0)
            emit_khT(0, 1)
            emit_qhT(0, 0)
            for kt in range(6):
                emit_vh(kt)

            # PE filler queues per (pair, q-block). Deps: kv group g feeds
            # khT columns g*256.. and vh[2g..2g+1]; khT(0, nb) is consumed by
            # scores(kt=4nb..) of block (0, 0); opart(t, nt) needs norm(t,
            # nt//4) which runs at the start of the next block after that.
            fillers = {
                (0, 0): ([(lambda g=g, h=h: tr_kv(g, range(3 * h, 3 * h + 3)))
                          for g in (4, 5) for h in (0, 1)]
                         + [lambda: emit_khT(0, 2)]
                         + [(lambda kt=kt: emit_vh(kt)) for kt in (6, 7)]
                         + [(lambda g=g, h=h: tr_kv(g, range(3 * h, 3 * h + 3)))
                            for g in (6, 7) for h in (0, 1)]
                         + [lambda: emit_khT(0, 3)]
                         + [(lambda kt=kt: emit_vh(kt)) for kt in range(8, KT)]
                         + [lambda: emit_qhT(0, 1)]),
                (0, 1): ([(lambda nb=nb: emit_khT(1, nb)) for nb in range(4)]
                         + [(lambda nb=nb: emit_qhT(1, nb)) for nb in range(QB)]),
                (1, 0): ([(lambda nt=nt: emit_opart(0, nt)) for nt in range(4)]
                         + [(lambda nb=nb: emit_khT(2, nb)) for nb in range(2)]),
                (1, 1): ([(lambda nb=nb: emit_khT(2, nb)) for nb in range(2, 4)]
                         + [(lambda nb=nb: emit_qhT(2, nb)) for nb in range(QB)]
                         + [(lambda nt=nt: emit_opart(0, nt)) for nt in range(4, 6)]),
                (2, 0): ([(lambda nt=nt: emit_opart(0, nt)) for nt in range(6, 8)]
                         + [(lambda nb=nb: emit_khT(3, nb)) for nb in range(2)]
                         + [(lambda nt=nt: emit_opart(1, nt)) for nt in range(2)]),
                (2, 1): ([(lambda nb=nb: emit_khT(3, nb)) for nb in range(2, 4)]
                         + [(lambda nb=nb: emit_qhT(3, nb)) for nb in range(QB)]
                         + [(lambda nt=nt: emit_opart(1, nt)) for nt in range(2, 4)]),
                (3, 0): ([(lambda nt=nt: emit_opart(1, nt)) for nt in range(4, 8)]
                         + [(lambda nt=nt: emit_opart(2, nt)) for nt in range(2)]),
                (3, 1): ([(lambda nt=nt: emit_opart(2, nt)) for nt in range(2, 8)]
                         + [(lambda nt=nt: emit_final(nt)) for nt in range(2)]),
            }

            carry = [None]

            def make_norm(t, qb, pvA, pvB):
                def emit():
                    qs = slice(qb * 512, (qb + 1) * 512)
                    dsb = norm_pool.tile([1, 1024], F32R, tag="nrm", name="dsb")
                    nc.vector.tensor_copy(dsb[0:1, 0:512], pvA[DH:DA, :])
                    nc.vector.tensor_copy(dsb[0:1, 512:1024], pvB[DH:DA, :])
                    dba = ps_mm.tile([64, 512], F32, tag="mm", name="dba")
                    dbb = ps_mm.tile([64, 512], F32, tag="mm", name="dbb")
                    nc.tensor.matmul(dba[:], ones1[:], dsb[0:1, 0:512],
                                     start=True, stop=True)
                    nc.tensor.matmul(dbb[:], ones1[:], dsb[0:1, 512:1024],
                                     start=True, stop=True)
                    rb = norm_pool.tile([64, 1024], F32, tag="nrm", name="rb")
                    nc.vector.reciprocal_approx_fast(rb[:, 0:512], dba[:])
                    nc.vector.reciprocal_approx_fast(rb[:, 512:1024], dbb[:])
                    nc.vector.tensor_mul(attnT[t][0:64, qs],
                                         pvA[0:DH, :], rb[:, 0:512])
                    nc.vector.tensor_mul(attnT[t][64:128, qs],
                                         pvB[0:DH, :], rb[:, 512:1024])
                return emit

            # ---- attention, software-pipelined [scores(k+1) | filler | PV(k)]
            for t in range(PAIRS):
                hA, hB = 2 * t, 2 * t + 1
                for qb in range(QB):
                    qs = slice(qb * 512, (qb + 1) * 512)
                    todo = fillers[(t, qb)]
                    fi = 0

                    def emit_scores(kt):
                        ks = slice(kt * 128, (kt + 1) * 128)
                        sc = ps_sc.tile([128, 1024], F32, tag="sc", name="sc")
                        nc.tensor.matmul(
                            sc[:, 0:512],
                            khT[t][0:64, ks], qhT[t][0:64, qs],
                            start=True, stop=True, tile_position=(0, 0))
                        nc.tensor.matmul(
                            sc[:, 512:1024],
                            khT[t][64:128, ks], qhT[t][64:128, qs],
                            start=True, stop=True, tile_position=(64, 0))
                        ex = exps_pool.tile([128, 1024], BF16, tag="exp", name="ex")
                        nc.scalar.activation(ex[:], sc[:], EXP,
                                             scale=float(DH) ** -0.5)
                        return ex

                    # depth-2 software pipeline: PV(k) trails scores(k) by
                    # two iterations; the last two PVs + normalization of this
                    # block carry into the next block's prologue so ScalarE is
                    # never starved at block boundaries.
                    exq = [emit_scores(0)]
                    if carry[0]:
                        carry[0][0]()      # PV(14) of the previous block
                    exq.append(emit_scores(1))
                    if carry[0]:
                        carry[0][1]()      # PV(15) of the previous block
                        carry[0][2]()      # normalization (frees old pv tiles)
                        carry[0] = None
                    pvA = ps_pv.tile([DA, 512], F32, tag="pv", name="pvA")
                    pvB = ps_pv.tile([DA, 512], F32, tag="pv", name="pvB")

                    def mk_pv(kt, ex, pvA=pvA, pvB=pvB, hA=hA, hB=hB):
                        def emit():
                            nc.tensor.matmul(pvA[:], vh[kt][:, hA, :],
                                             ex[:, 0:512],
                                             start=(kt == 0), stop=(kt == KT - 1))
                            nc.tensor.matmul(pvB[:], vh[kt][:, hB, :],
                                             ex[:, 512:1024],
                                             start=(kt == 0), stop=(kt == KT - 1))
                        return emit

                    per_iter = 2 if (t, qb) == (0, 0) else 1
                    for kt in range(2, KT):
                        exq.append(emit_scores(kt))
                        for _ in range(per_iter):
                            if fi < len(todo) and (per_iter == 2 or kt % 2 == 0):
                                todo[fi]()
                                fi += 1
                        mk_pv(kt - 2, exq[kt - 2])()
                    mk_pv(KT - 2, exq[KT - 2])()
                    while fi < len(todo):
                        todo[fi]()
                        fi += 1
                    carry[0] = [mk_pv(KT - 1, exq[KT - 1]), lambda: None,
                                make_norm(t, qb, pvA, pvB)]
                if t == 0:
                    _es.close()   # free the kv staging + weight staging SBUF
            carry[0][0]()
            carry[0][2]()

            # ---- remaining final out-projection rows ----
            for nt in range(2, NT):
                emit_final(nt)

    nc.compile()
    return nc


def kernel(q, kv, Wq, Wk, Wv, Wo, bo):
    from concourse.bass_utils import run_bass_kernel_spmd

    q = np.asarray(q, dtype=np.float32)
    kv = np.asarray(kv, dtype=np.float32)
    Wq = np.ascontiguousarray(np.asarray(Wq, dtype=np.float32))
    Wk = np.ascontiguousarray(np.asarray(Wk, dtype=np.float32))
    Wv = np.ascontiguousarray(np.asarray(Wv, dtype=np.float32))
    Wo = np.ascontiguousarray(np.asarray(Wo, dtype=np.float32))
    bo = np.ascontiguousarray(np.asarray(bo, dtype=np.float32))

    if "nc" not in _cache:
        _cache["nc"] = _build()
    nc = _cache["nc"]

    in_maps = []
    for c in range(N_CORES):
        b, h = c // 2, c % 2
        in_maps.append({
            "q": np.ascontiguousarray(q[b, h * NQ:(h + 1) * NQ]),
            "kv": np.ascontiguousarray(kv[b]),
            "Wq": Wq, "Wk": Wk, "Wv": Wv, "Wo": Wo, "bo": bo,
        })
    res = run_bass_kernel_spmd(nc, in_maps, core_ids=list(range(N_CORES)))
    out = np.empty((B, NQ_FULL, DQ), dtype=np.float32)
    for c in range(N_CORES):
        b, h = c // 2, c % 2
        out[b, h * NQ:(h + 1) * NQ] = res.results[c]["out"]
    return out
